# revision 13
# baseline (speedup 1.0000x reference)
"""BlurDownsample (depthwise 4x4 FIR + 2x downsample) on 8 TRN2 NeuronCores.

Contract: kernel(x, f) takes the FULL inputs
    x: [16, 128, 256, 256] float32,  f: [4, 4] float32
and returns the FULL output [16, 128, 128, 128] float32, matching
    upfirdn2d(x, f, down=2, padding=(1, 1), flip_filter=False):
    out[n,c,oy,ox] = sum_{dy,dx in 0..3} f[3-dy, 3-dx] * xpad[2oy+dy, 2ox+dx]
with xpad zero-padded by 1 on every spatial edge.

Sharding: pure data-parallel over the batch — core k processes
x[2k:2k+2]; filter-derived constants are replicated.

Per-core strategy (v3 — SDMA-descriptor + HBM-traffic optimized):
  * Host-side, the flipped filter g = flip(f) is factored by SVD into
    R separable terms g = sum_r ah_r (x) bw_r  (R=1 for the
    outer-product filter the model uses).  Only the H-direction runs
    on the Tensor engine; the W-direction is a 4-tap combine on the
    Scalar + Vector engines.  This cuts Tensor-engine streaming 4x
    vs. the banded-matmul-per-filter-column approach.
  * x is converted to bf16 on the host and uploaded as
    [N, C, 128, 512]: HBM read traffic halves (the 2e-2 rel-err gate
    dwarfs bf16's ~2^-9 rounding), every DMA piece is a contiguous
    row pair, and no in-flight cast is needed so loads ride the
    fast HWDGE (sync-engine) path.  SDMA descriptor handling — not
    HBM bandwidth — capped the previous version.
  * The H-FIR+downsample is polyphase banded matmuls in bf16 over
    row-pair partitions: for row parity e, band B_e[p, oh] =
    ah[2p+e-2oh+1] contracts row pairs p, accumulating
    mid[oh, c2, w] in PSUM (2 channels per matmul, rhs free = 512).
    Zero padding in H is implicit in the bands (built host-side).
  * W-combine per channel quad: out[ox] = sum_dx bw[dx]*mid[2ox-1+dx]
    = one Scalar-engine scaled copy (dx=1, full range) plus three
    Vector scalar_tensor_tensor fused multiply-adds (dx=2 full range,
    dx=0/dx=3 edge-clipped), taps as fp32 per-partition SBUF scalars.
  * Stores (fp32) use the scalar-engine HWDGE ring, separate from
    the load ring.
"""

from contextlib import ExitStack

import numpy as np

import concourse.tile as tile
from concourse import bacc, mybir
from concourse.bass_utils import run_bass_kernel_spmd

F32 = mybir.dt.float32
BF16 = mybir.dt.bfloat16

N_CORES = 8
FW = 4  # filter size


def _build_blur_program(nc, N, C, H, W, R):
    OH, OW = H // 2, W // 2
    P = H // 2              # row pairs = SBUF partitions for the contraction
    W2 = 2 * W              # elements per partition row-pair
    CG = min(C, 16)         # channels per load/store group
    QC = 2                  # channels per matmul (PSUM bank: N*4B <= 2KB)
    JJ = min(CG // QC, max(1, 8 // (2 * R)))  # matmul quads per PSUM tile
    assert C % CG == 0 and CG % QC == 0 and P == 128 and W == 256

    x_ap = nc.dram_tensor("x", [N, C, P, W2], BF16, kind="ExternalInput").ap()
    bh_ap = nc.dram_tensor("bh", [R, 2, P, OH], BF16, kind="ExternalInput").ap()
    wt_ap = nc.dram_tensor("wt", [P, 4 * R], F32, kind="ExternalInput").ap()
    out_ap = nc.dram_tensor("out", [N, C, OH, OW], F32, kind="ExternalOutput").ap()

    with tile.TileContext(nc) as tc, ExitStack() as ctx:
        const_pool = ctx.enter_context(tc.tile_pool(name="const", bufs=1))
        x_pool = ctx.enter_context(tc.tile_pool(name="xt", bufs=3))
        acc_pool = ctx.enter_context(tc.tile_pool(name="acc", bufs=2))
        psum_pool = ctx.enter_context(tc.tile_pool(name="mid", bufs=1, space="PSUM"))

        # ---- one-time setup: load bands + taps ----
        bh_sb = const_pool.tile([P, R, 2, OH], BF16, tag="bh")
        for r in range(R):
            for e in range(2):
                nc.sync.dma_start(out=bh_sb[:, r, e, :], in_=bh_ap[r, e])
        wt_sb = const_pool.tile([P, 4 * R], F32, tag="wt")
        nc.sync.dma_start(out=wt_sb[:, :], in_=wt_ap)

        # ---- main loop: groups of CG channels ----
        for n in range(N):
            for c0 in range(0, C, CG):
                xt = x_pool.tile([P, CG, W2], BF16, tag="xt")
                nc.sync.dma_start(  # 1 KiB contiguous pieces (row pairs)
                    out=xt[:, :, :],
                    in_=x_ap[n, c0 : c0 + CG].rearrange("c p w -> p c w"),
                )
                acc = acc_pool.tile([OH, CG // QC, QC, OW], F32, tag="acc")
                # Quads are processed in PAIRS with the combine chains
                # interleaved across the pair so no engine queue stalls
                # head-of-line: each op's dependency completed 2 slots
                # earlier.  Tensor keeps ~2 quads of lookahead, which
                # keeps its idle gaps under the ~3.4us HAM re-throttle
                # window (cold PE halves matmul throughput).
                NQ = CG // (QC * JJ)
                assert NQ % 2 == 0
                for jq0 in range(0, NQ, 2):
                    quads = (jq0, jq0 + 1)
                    mids = {}
                    for jq in quads:
                        for r in range(R):
                            mid = psum_pool.tile(
                                [OH, JJ, QC, W], F32, tag=f"mid{jq % 2}_{r}"
                            )
                            mids[(jq, r)] = mid
                            for e in range(2):
                                for jj in range(JJ):
                                    c1 = QC * (JJ * jq + jj)
                                    nc.tensor.matmul(
                                        mid[:, jj, :, :],
                                        lhsT=bh_sb[:, r, e, :],
                                        rhs=xt[
                                            :, c1 : c1 + QC, e * W : (e + 1) * W
                                        ],
                                        start=(e == 0),
                                        stop=(e == 1),
                                    )
                    # W-combine: out[ox] += sum_dx bw[dx]*mid[2ox-1+dx]
                    for r in range(R):
                        # dx=1: iw = 2ox, full range — Scalar engine init
                        for jq in quads:
                            js = slice(JJ * jq, JJ * (jq + 1))
                            a_full = acc[:, js, :, :]
                            mid = mids[(jq, r)]
                            if r == 0:
                                nc.scalar.mul(
                                    a_full,
                                    mid[:, :, :, 0:W:2],
                                    wt_sb[:, 4 * r + 1 : 4 * r + 2],
                                )
                            else:
                                nc.vector.scalar_tensor_tensor(
                                    a_full,
                                    mid[:, :, :, 0:W:2],
                                    wt_sb[:, 4 * r + 1 : 4 * r + 2],
                                    a_full,
                                    op0=mybir.AluOpType.mult,
                                    op1=mybir.AluOpType.add,
                                )
                        # dx=2 (full), dx=0 (ox>=1), dx=3 (ox<=OW-2) — Vector
                        for tap, lo, hi, alo, ahi in (
                            (2, 1, W, 0, OW),
                            (0, 1, W - 2, 1, OW),
                            (3, 2, W - 1, 0, OW - 1),
                        ):
                            for jq in quads:
                                js = slice(JJ * jq, JJ * (jq + 1))
                                at = acc[:, js, :, alo:ahi]
                                mid = mids[(jq, r)]
                                nc.vector.scalar_tensor_tensor(
                                    at,
                                    mid[:, :, :, lo:hi:2],
                                    wt_sb[:, 4 * r + tap : 4 * r + tap + 1],
                                    at,
                                    op0=mybir.AluOpType.mult,
                                    op1=mybir.AluOpType.add,
                                )
                nc.scalar.dma_start(
                    out=out_ap[n, c0 : c0 + CG].rearrange("c oh ow -> oh c ow"),
                    in_=acc[:, :, :, :].rearrange("p a b w -> p (a b) w"),
                )
    return nc


def _factor_filter(f):
    """Factor the flipped filter into R separable (ah, bw) term pairs."""
    g = np.flip(np.asarray(f, dtype=np.float64))
    U, s, Vt = np.linalg.svd(g)
    if s[0] <= 0.0:
        return 0, None, None
    R = int(np.sum(s > s[0] * 1e-4))
    ah = (U[:, :R] * np.sqrt(s[:R])).astype(np.float32)        # [4, R]
    bw = (Vt[:R, :].T * np.sqrt(s[:R])).astype(np.float32)     # [4, R]
    return R, ah, bw


def _build_inputs(ah, bw, P, OH, R):
    bh = np.zeros((R, 2, P, OH), dtype=np.float32)
    for r in range(R):
        for e in range(2):
            for d in range(-2, 3):  # oh = p - d; band is narrow
                dy = 2 * d + e + 1
                if 0 <= dy < FW:
                    idx = np.arange(max(0, d), min(P, OH + d))
                    bh[r, e, idx, idx - d] = ah[dy, r]
    wt = np.tile(bw.T.reshape(1, 4 * R), (P, 1)).astype(np.float32)
    return bh, wt


_PROGRAM_CACHE = {}


def _get_program(shape, R):
    key = (shape, R)
    if key not in _PROGRAM_CACHE:
        N, C, H, W = shape
        nb = N // N_CORES
        nc = bacc.Bacc(
            "TRN2", target_bir_lowering=False, debug=False, num_devices=N_CORES
        )
        _build_blur_program(nc, nb, C, H, W, R)
        nc.compile()
        _PROGRAM_CACHE[key] = nc
    return _PROGRAM_CACHE[key]


def _run(x, f, trace=False, tmpdir=None):
    x = np.ascontiguousarray(x, dtype=np.float32)
    f = np.ascontiguousarray(f, dtype=np.float32)
    N, C, H, W = x.shape
    OH, OW = H // 2, W // 2
    assert N % N_CORES == 0, f"batch {N} not divisible by {N_CORES} cores"
    nb = N // N_CORES

    R, ah, bw = _factor_filter(f)
    if R == 0:
        return np.zeros((N, C, OH, OW), dtype=np.float32), None
    bh, wt = _build_inputs(ah, bw, H // 2, OH, R)

    nc = _get_program((N, C, H, W), R)
    np_bf16 = mybir.dt.np(BF16)
    xv = np.ascontiguousarray(
        x.reshape(N, C, H // 2, 2 * W).astype(np_bf16)
    )
    bhv = bh.astype(np_bf16)
    in_maps = [
        {"x": xv[k * nb : (k + 1) * nb], "bh": bhv, "wt": wt}
        for k in range(N_CORES)
    ]
    res = run_bass_kernel_spmd(
        nc, in_maps, core_ids=list(range(N_CORES)), trace=trace, tmpdir=tmpdir
    )
    out = np.concatenate(
        [res.results[k]["out"] for k in range(N_CORES)], axis=0
    )
    return out, res


def kernel(x, f):
    out, _ = _run(x, f)
    return out


# revision 20
# speedup vs baseline: 1.2009x; 1.2009x over previous
"""BlurDownsample (depthwise 4x4 FIR + 2x downsample) on 8 TRN2 NeuronCores.

Contract: kernel(x, f) takes the FULL inputs
    x: [16, 128, 256, 256] float32,  f: [4, 4] float32
and returns the FULL output [16, 128, 128, 128] float32, matching
    upfirdn2d(x, f, down=2, padding=(1, 1), flip_filter=False):
    out[n,c,oy,ox] = sum_{dy,dx in 0..3} f[3-dy, 3-dx] * xpad[2oy+dy, 2ox+dx]
with xpad zero-padded by 1 on every spatial edge.

Sharding: pure data-parallel over the batch — core k processes
x[2k:2k+2]; filter-derived constants are replicated.

Per-core strategy (v3 — SDMA-descriptor + HBM-traffic optimized):
  * Host-side, the flipped filter g = flip(f) is factored by SVD into
    R separable terms g = sum_r ah_r (x) bw_r  (R=1 for the
    outer-product filter the model uses).  Only the H-direction runs
    on the Tensor engine; the W-direction is a 4-tap combine on the
    Scalar + Vector engines.  This cuts Tensor-engine streaming 4x
    vs. the banded-matmul-per-filter-column approach.
  * x is converted to bf16 on the host and uploaded as
    [N, C, 128, 512]: HBM read traffic halves (the 2e-2 rel-err gate
    dwarfs bf16's ~2^-9 rounding), every DMA piece is a contiguous
    row pair, and no in-flight cast is needed so loads ride the
    fast HWDGE (sync-engine) path.  SDMA descriptor handling — not
    HBM bandwidth — capped the previous version.
  * The H-FIR+downsample is polyphase banded matmuls in bf16 over
    row-pair partitions: for row parity e, band B_e[p, oh] =
    ah[2p+e-2oh+1] contracts row pairs p, accumulating
    mid[oh, c2, w] in PSUM (2 channels per matmul, rhs free = 512).
    Zero padding in H is implicit in the bands (built host-side).
  * W-combine per channel quad: out[ox] = sum_dx bw[dx]*mid[2ox-1+dx]
    = one Scalar-engine scaled copy (dx=1, full range) plus three
    Vector scalar_tensor_tensor fused multiply-adds (dx=2 full range,
    dx=0/dx=3 edge-clipped), taps as fp32 per-partition SBUF scalars.
  * Stores (fp32) use the scalar-engine HWDGE ring, separate from
    the load ring.
"""

from contextlib import ExitStack

import numpy as np

import concourse.tile as tile
from concourse import bacc, mybir
from concourse.bass_utils import run_bass_kernel_spmd

F32 = mybir.dt.float32
BF16 = mybir.dt.bfloat16

N_CORES = 8
FW = 4  # filter size


def _build_blur_program(nc, N, C, H, W, R, fast):
    OH, OW = H // 2, W // 2
    P = H // 2              # row pairs = SBUF partitions for the contraction
    W2 = 2 * W              # elements per partition row-pair
    CG = min(C, 16)         # channels per load/store group
    QC = 2                  # channels per matmul (PSUM bank: N*4B <= 2KB)
    JJ = min(CG // QC, max(1, 8 // (2 * R)))  # matmul quads per PSUM tile
    assert C % CG == 0 and CG % QC == 0 and P == 128 and W == 256

    x_ap = nc.dram_tensor("x", [N, C, P, W2], BF16, kind="ExternalInput").ap()
    bh_ap = nc.dram_tensor("bh", [R, 2, P, OH], BF16, kind="ExternalInput").ap()
    wt_ap = nc.dram_tensor("wt", [P, 6 * R], F32, kind="ExternalInput").ap()
    out_ap = nc.dram_tensor("out", [N, C, OH, OW], F32, kind="ExternalOutput").ap()

    with tile.TileContext(nc) as tc, ExitStack() as ctx:
        const_pool = ctx.enter_context(tc.tile_pool(name="const", bufs=1))
        x_pool = ctx.enter_context(tc.tile_pool(name="xt", bufs=3))
        acc_pool = ctx.enter_context(tc.tile_pool(name="acc", bufs=2))
        eo_pool = ctx.enter_context(tc.tile_pool(name="eo", bufs=2))
        psum_pool = ctx.enter_context(tc.tile_pool(name="mid", bufs=2, space="PSUM"))

        # ---- one-time setup: load bands + taps ----
        bh_sb = const_pool.tile([P, R, 2, OH], BF16, tag="bh")
        for r in range(R):
            for e in range(2):
                nc.sync.dma_start(out=bh_sb[:, r, e, :], in_=bh_ap[r, e])
        wt_sb = const_pool.tile([P, 6 * R], F32, tag="wt")
        nc.sync.dma_start(out=wt_sb[:, :], in_=wt_ap)

        def wtc(i):
            return wt_sb[:, i : i + 1]

        # ---- main loop: groups of CG channels ----
        for n in range(N):
            for c0 in range(0, C, CG):
                xt = x_pool.tile([P, CG, W2], BF16, tag="xt")
                nc.sync.dma_start(  # 1 KiB contiguous pieces (row pairs)
                    out=xt[:, :, :],
                    in_=x_ap[n, c0 : c0 + CG].rearrange("c p w -> p c w"),
                )
                acc = acc_pool.tile([OH, CG // QC, QC, OW], F32, tag="acc")
                for jq in range(CG // (QC * JJ)):
                    js = slice(JJ * jq, JJ * (jq + 1))
                    a_full = acc[:, js, :, :]
                    a0 = acc[:, js, :, 1:OW]
                    a3 = acc[:, js, :, 0 : OW - 1]
                    for r in range(R):
                        mid = psum_pool.tile([OH, JJ, QC, W], F32, tag=f"mid{r}")
                        for e in range(2):
                            for jj in range(JJ):
                                c1 = QC * (JJ * jq + jj)
                                nc.tensor.matmul(
                                    mid[:, jj, :, :],
                                    lhsT=bh_sb[:, r, e, :],
                                    rhs=xt[:, c1 : c1 + QC, e * W : (e + 1) * W],
                                    start=(e == 0),
                                    stop=(e == 1),
                                )
                        # W-combine: out[ox] += sum_dx bw[dx]*mid[2ox-1+dx]
                        # with E[k]=mid[2k] (taps 1,3), O[k]=mid[2k+1]
                        # (taps 2,0).
                        if fast[r]:
                            # Scalar engine drains PSUM fast (frees it for
                            # the tensor engine) into scaled polyphases:
                            #   esb = bw1*E, osb = bw2*O.
                            # Vector then combines ALL-SBUF, which enables
                            # the DVE 2x_2p perf mode (fp32 at 2 elem/cyc):
                            #   acc  = esb + osb        (taps 1 and 2)
                            #   acc[1:]   += (bw0/bw2)*osb[:-1]   (tap 0)
                            #   acc[:-1]  += (bw3/bw1)*esb[1:]    (tap 3)
                            esb = eo_pool.tile([OH, JJ, QC, OW], F32, tag="esb")
                            osb = eo_pool.tile([OH, JJ, QC, OW], F32, tag="osb")
                            nc.scalar.mul(
                                esb[:, :, :, :], mid[:, :, :, 0:W:2], wtc(6 * r + 1)
                            )
                            nc.scalar.mul(
                                osb[:, :, :, :], mid[:, :, :, 1:W:2], wtc(6 * r + 2)
                            )
                            if r == 0:
                                nc.vector.scalar_tensor_tensor(
                                    a_full,
                                    esb[:, :, :, :],
                                    1.0,
                                    osb[:, :, :, :],
                                    op0=mybir.AluOpType.mult,
                                    op1=mybir.AluOpType.add,
                                )
                            else:
                                nc.vector.scalar_tensor_tensor(
                                    a_full,
                                    esb[:, :, :, :],
                                    1.0,
                                    a_full,
                                    op0=mybir.AluOpType.mult,
                                    op1=mybir.AluOpType.add,
                                )
                                nc.vector.scalar_tensor_tensor(
                                    a_full,
                                    osb[:, :, :, :],
                                    1.0,
                                    a_full,
                                    op0=mybir.AluOpType.mult,
                                    op1=mybir.AluOpType.add,
                                )
                            nc.vector.scalar_tensor_tensor(
                                a0,
                                osb[:, :, :, 0 : OW - 1],
                                wtc(6 * r + 4),
                                a0,
                                op0=mybir.AluOpType.mult,
                                op1=mybir.AluOpType.add,
                            )
                            nc.vector.scalar_tensor_tensor(
                                a3,
                                esb[:, :, :, 1:OW],
                                wtc(6 * r + 5),
                                a3,
                                op0=mybir.AluOpType.mult,
                                op1=mybir.AluOpType.add,
                            )
                        else:
                            # Degenerate inner taps: combine straight from
                            # PSUM (no ratio trick available).
                            if r == 0:
                                nc.scalar.mul(
                                    a_full, mid[:, :, :, 0:W:2], wtc(6 * r + 1)
                                )
                            else:
                                nc.vector.scalar_tensor_tensor(
                                    a_full,
                                    mid[:, :, :, 0:W:2],
                                    wtc(6 * r + 1),
                                    a_full,
                                    op0=mybir.AluOpType.mult,
                                    op1=mybir.AluOpType.add,
                                )
                            for tap, lo, hi, at in (
                                (2, 1, W, a_full),
                                (0, 1, W - 2, a0),
                                (3, 2, W - 1, a3),
                            ):
                                nc.vector.scalar_tensor_tensor(
                                    at,
                                    mid[:, :, :, lo:hi:2],
                                    wtc(6 * r + tap),
                                    at,
                                    op0=mybir.AluOpType.mult,
                                    op1=mybir.AluOpType.add,
                                )
                nc.scalar.dma_start(
                    out=out_ap[n, c0 : c0 + CG].rearrange("c oh ow -> oh c ow"),
                    in_=acc[:, :, :, :].rearrange("p a b w -> p (a b) w"),
                )
    return nc


def _factor_filter(f):
    """Factor the flipped filter into R separable (ah, bw) term pairs."""
    g = np.flip(np.asarray(f, dtype=np.float64))
    U, s, Vt = np.linalg.svd(g)
    if s[0] <= 0.0:
        return 0, None, None
    R = int(np.sum(s > s[0] * 1e-4))
    ah = (U[:, :R] * np.sqrt(s[:R])).astype(np.float32)        # [4, R]
    bw = (Vt[:R, :].T * np.sqrt(s[:R])).astype(np.float32)     # [4, R]
    return R, ah, bw


def _build_inputs(ah, bw, P, OH, R):
    bh = np.zeros((R, 2, P, OH), dtype=np.float32)
    for r in range(R):
        for e in range(2):
            for d in range(-2, 3):  # oh = p - d; band is narrow
                dy = 2 * d + e + 1
                if 0 <= dy < FW:
                    idx = np.arange(max(0, d), min(P, OH + d))
                    bh[r, e, idx, idx - d] = ah[dy, r]
    # wt row per r: [bw0, bw1, bw2, bw3, bw0/bw2, bw3/bw1] (ratios only
    # used on the fast path, which requires |bw1|,|bw2| >> 0)
    fast = []
    wt = np.zeros((R, 6), dtype=np.float64)
    for r in range(R):
        b = bw[:, r].astype(np.float64)
        mx = np.abs(b).max()
        ok = mx > 0 and min(abs(b[1]), abs(b[2])) > 1e-4 * mx
        fast.append(bool(ok))
        wt[r, 0:4] = b
        if ok:
            wt[r, 4] = b[0] / b[2]
            wt[r, 5] = b[3] / b[1]
    wt = np.tile(wt.reshape(1, 6 * R).astype(np.float32), (P, 1))
    return bh, wt, tuple(fast)


_PROGRAM_CACHE = {}


def _get_program(shape, R, fast):
    key = (shape, R, fast)
    if key not in _PROGRAM_CACHE:
        N, C, H, W = shape
        nb = N // N_CORES
        nc = bacc.Bacc(
            "TRN2", target_bir_lowering=False, debug=False, num_devices=N_CORES
        )
        _build_blur_program(nc, nb, C, H, W, R, fast)
        nc.compile()
        _PROGRAM_CACHE[key] = nc
    return _PROGRAM_CACHE[key]


def _run(x, f, trace=False, tmpdir=None):
    x = np.ascontiguousarray(x, dtype=np.float32)
    f = np.ascontiguousarray(f, dtype=np.float32)
    N, C, H, W = x.shape
    OH, OW = H // 2, W // 2
    assert N % N_CORES == 0, f"batch {N} not divisible by {N_CORES} cores"
    nb = N // N_CORES

    R, ah, bw = _factor_filter(f)
    if R == 0:
        return np.zeros((N, C, OH, OW), dtype=np.float32), None
    bh, wt, fast = _build_inputs(ah, bw, H // 2, OH, R)

    nc = _get_program((N, C, H, W), R, fast)
    np_bf16 = mybir.dt.np(BF16)
    xv = np.ascontiguousarray(
        x.reshape(N, C, H // 2, 2 * W).astype(np_bf16)
    )
    bhv = bh.astype(np_bf16)
    in_maps = [
        {"x": xv[k * nb : (k + 1) * nb], "bh": bhv, "wt": wt}
        for k in range(N_CORES)
    ]
    res = run_bass_kernel_spmd(
        nc, in_maps, core_ids=list(range(N_CORES)), trace=trace, tmpdir=tmpdir
    )
    out = np.concatenate(
        [res.results[k]["out"] for k in range(N_CORES)], axis=0
    )
    return out, res


def kernel(x, f):
    out, _ = _run(x, f)
    return out


# revision 27
# speedup vs baseline: 1.2682x; 1.0560x over previous
"""BlurDownsample (depthwise 4x4 FIR + 2x downsample) on 8 TRN2 NeuronCores.

Contract: kernel(x, f) takes the FULL inputs
    x: [16, 128, 256, 256] float32,  f: [4, 4] float32
and returns the FULL output [16, 128, 128, 128] float32, matching
    upfirdn2d(x, f, down=2, padding=(1, 1), flip_filter=False):
    out[n,c,oy,ox] = sum_{dy,dx in 0..3} f[3-dy, 3-dx] * xpad[2oy+dy, 2ox+dx]
with xpad zero-padded by 1 on every spatial edge.

Sharding: pure data-parallel over the batch — core k processes
x[2k:2k+2]; filter-derived constants are replicated.

Per-core strategy (v3 — SDMA-descriptor + HBM-traffic optimized):
  * Host-side, the flipped filter g = flip(f) is factored by SVD into
    R separable terms g = sum_r ah_r (x) bw_r  (R=1 for the
    outer-product filter the model uses).  Only the H-direction runs
    on the Tensor engine; the W-direction is a 4-tap combine on the
    Scalar + Vector engines.  This cuts Tensor-engine streaming 4x
    vs. the banded-matmul-per-filter-column approach.
  * x is converted to bf16 on the host and uploaded as
    [N, C, 128, 512]: HBM read traffic halves (the 2e-2 rel-err gate
    dwarfs bf16's ~2^-9 rounding), every DMA piece is a contiguous
    row pair, and no in-flight cast is needed so loads ride the
    fast HWDGE (sync-engine) path.  SDMA descriptor handling — not
    HBM bandwidth — capped the previous version.
  * The H-FIR+downsample is polyphase banded matmuls in bf16 over
    row-pair partitions: for row parity e, band B_e[p, oh] =
    ah[2p+e-2oh+1] contracts row pairs p, accumulating
    mid[oh, c2, w] in PSUM (2 channels per matmul, rhs free = 512).
    Zero padding in H is implicit in the bands (built host-side).
  * W-combine per channel quad: out[ox] = sum_dx bw[dx]*mid[2ox-1+dx]
    = one Scalar-engine scaled copy (dx=1, full range) plus three
    Vector scalar_tensor_tensor fused multiply-adds (dx=2 full range,
    dx=0/dx=3 edge-clipped), taps as fp32 per-partition SBUF scalars.
  * Stores (fp32) use the scalar-engine HWDGE ring, separate from
    the load ring.
"""

from contextlib import ExitStack

import numpy as np

import concourse.tile as tile
from concourse import bacc, mybir
from concourse.bass_utils import run_bass_kernel_spmd

F32 = mybir.dt.float32
BF16 = mybir.dt.bfloat16

N_CORES = 8
FW = 4  # filter size


def _build_blur_program(nc, N, C, H, W, R, fast):
    OH, OW = H // 2, W // 2
    P = H // 2              # row pairs = SBUF partitions for the contraction
    W2 = 2 * W              # elements per partition row-pair
    CG = min(C, 32)         # channels per load/store group
    QC = 2                  # channels per matmul (PSUM bank: N*4B <= 2KB)
    JJ = min(CG // QC, max(1, 8 // (2 * R)))  # matmul quads per PSUM tile
    assert C % CG == 0 and CG % QC == 0 and P == 128 and W == 256

    x_ap = nc.dram_tensor("x", [N, P, C, W2], BF16, kind="ExternalInput").ap()
    bh_ap = nc.dram_tensor("bh", [R, 2, P, OH], BF16, kind="ExternalInput").ap()
    wt_ap = nc.dram_tensor("wt", [P, 6 * R], F32, kind="ExternalInput").ap()
    out_ap = nc.dram_tensor("out", [N, C, OH, OW], F32, kind="ExternalOutput").ap()

    with tile.TileContext(nc) as tc, ExitStack() as ctx:
        const_pool = ctx.enter_context(tc.tile_pool(name="const", bufs=1))
        x_pool = ctx.enter_context(tc.tile_pool(name="xt", bufs=3))
        acc_pool = ctx.enter_context(tc.tile_pool(name="acc", bufs=2))
        eo_pool = ctx.enter_context(tc.tile_pool(name="eo", bufs=4))
        psum_pool = ctx.enter_context(tc.tile_pool(name="mid", bufs=2, space="PSUM"))

        # ---- one-time setup: load bands + taps ----
        bh_sb = const_pool.tile([P, R, 2, OH], BF16, tag="bh")
        for r in range(R):
            for e in range(2):
                nc.sync.dma_start(out=bh_sb[:, r, e, :], in_=bh_ap[r, e])
        wt_sb = const_pool.tile([P, 6 * R], F32, tag="wt")
        nc.sync.dma_start(out=wt_sb[:, :], in_=wt_ap)

        def wtc(i):
            return wt_sb[:, i : i + 1]

        # ---- main loop: groups of CG channels ----
        for n in range(N):
            for c0 in range(0, C, CG):
                xt = x_pool.tile([P, CG, W2], BF16, tag="xt")
                nc.sync.dma_start(  # host-transposed layout: CG KiB pieces
                    out=xt[:, :, :],
                    in_=x_ap[n, :, c0 : c0 + CG, :],
                )
                acc = acc_pool.tile([OH, CG // QC, QC, OW], F32, tag="acc")
                for jq in range(CG // (QC * JJ)):
                    js = slice(JJ * jq, JJ * (jq + 1))
                    a_full = acc[:, js, :, :]
                    a0 = acc[:, js, :, 1:OW]
                    a3 = acc[:, js, :, 0 : OW - 1]
                    for r in range(R):
                        mid = psum_pool.tile([OH, JJ, QC, W], F32, tag=f"mid{r}")
                        for e in range(2):
                            for jj in range(JJ):
                                c1 = QC * (JJ * jq + jj)
                                nc.tensor.matmul(
                                    mid[:, jj, :, :],
                                    lhsT=bh_sb[:, r, e, :],
                                    rhs=xt[:, c1 : c1 + QC, e * W : (e + 1) * W],
                                    start=(e == 0),
                                    stop=(e == 1),
                                )
                        # W-combine: out[ox] += sum_dx bw[dx]*mid[2ox-1+dx]
                        # with E[k]=mid[2k] (taps 1,3), O[k]=mid[2k+1]
                        # (taps 2,0).
                        if fast[r]:
                            # Scalar engine drains PSUM fast (frees it for
                            # the tensor engine) into scaled polyphases:
                            #   esb = bw1*E, osb = bw2*O.
                            # Vector then combines ALL-SBUF, which enables
                            # the DVE 2x_2p perf mode (fp32 at 2 elem/cyc):
                            #   acc  = esb + osb        (taps 1 and 2)
                            #   acc[1:]   += (bw0/bw2)*osb[:-1]   (tap 0)
                            #   acc[:-1]  += (bw3/bw1)*esb[1:]    (tap 3)
                            esb = eo_pool.tile([OH, JJ, QC, OW], F32, tag="esb")
                            osb = eo_pool.tile([OH, JJ, QC, OW], F32, tag="osb")
                            nc.scalar.mul(
                                esb[:, :, :, :], mid[:, :, :, 0:W:2], wtc(6 * r + 1)
                            )
                            nc.scalar.mul(
                                osb[:, :, :, :], mid[:, :, :, 1:W:2], wtc(6 * r + 2)
                            )
                            if r == 0:
                                nc.vector.scalar_tensor_tensor(
                                    a_full,
                                    esb[:, :, :, :],
                                    1.0,
                                    osb[:, :, :, :],
                                    op0=mybir.AluOpType.mult,
                                    op1=mybir.AluOpType.add,
                                )
                            else:
                                nc.vector.scalar_tensor_tensor(
                                    a_full,
                                    esb[:, :, :, :],
                                    1.0,
                                    a_full,
                                    op0=mybir.AluOpType.mult,
                                    op1=mybir.AluOpType.add,
                                )
                                nc.vector.scalar_tensor_tensor(
                                    a_full,
                                    osb[:, :, :, :],
                                    1.0,
                                    a_full,
                                    op0=mybir.AluOpType.mult,
                                    op1=mybir.AluOpType.add,
                                )
                            nc.vector.scalar_tensor_tensor(
                                a3,
                                esb[:, :, :, 1:OW],
                                wtc(6 * r + 5),
                                a3,
                                op0=mybir.AluOpType.mult,
                                op1=mybir.AluOpType.add,
                            )
                            nc.vector.scalar_tensor_tensor(
                                a0,
                                osb[:, :, :, 0 : OW - 1],
                                wtc(6 * r + 4),
                                a0,
                                op0=mybir.AluOpType.mult,
                                op1=mybir.AluOpType.add,
                            )
                        else:
                            # Degenerate inner taps: combine straight from
                            # PSUM (no ratio trick available).
                            if r == 0:
                                nc.scalar.mul(
                                    a_full, mid[:, :, :, 0:W:2], wtc(6 * r + 1)
                                )
                            else:
                                nc.vector.scalar_tensor_tensor(
                                    a_full,
                                    mid[:, :, :, 0:W:2],
                                    wtc(6 * r + 1),
                                    a_full,
                                    op0=mybir.AluOpType.mult,
                                    op1=mybir.AluOpType.add,
                                )
                            for tap, lo, hi, at in (
                                (2, 1, W, a_full),
                                (0, 1, W - 2, a0),
                                (3, 2, W - 1, a3),
                            ):
                                nc.vector.scalar_tensor_tensor(
                                    at,
                                    mid[:, :, :, lo:hi:2],
                                    wtc(6 * r + tap),
                                    at,
                                    op0=mybir.AluOpType.mult,
                                    op1=mybir.AluOpType.add,
                                )
                nc.scalar.dma_start(
                    out=out_ap[n, c0 : c0 + CG].rearrange("c oh ow -> oh c ow"),
                    in_=acc[:, :, :, :].rearrange("p a b w -> p (a b) w"),
                )
    return nc


def _factor_filter(f):
    """Factor the flipped filter into R separable (ah, bw) term pairs."""
    g = np.flip(np.asarray(f, dtype=np.float64))
    U, s, Vt = np.linalg.svd(g)
    if s[0] <= 0.0:
        return 0, None, None
    R = int(np.sum(s > s[0] * 1e-4))
    ah = (U[:, :R] * np.sqrt(s[:R])).astype(np.float32)        # [4, R]
    bw = (Vt[:R, :].T * np.sqrt(s[:R])).astype(np.float32)     # [4, R]
    return R, ah, bw


def _build_inputs(ah, bw, P, OH, R):
    bh = np.zeros((R, 2, P, OH), dtype=np.float32)
    for r in range(R):
        for e in range(2):
            for d in range(-2, 3):  # oh = p - d; band is narrow
                dy = 2 * d + e + 1
                if 0 <= dy < FW:
                    idx = np.arange(max(0, d), min(P, OH + d))
                    bh[r, e, idx, idx - d] = ah[dy, r]
    # wt row per r: [bw0, bw1, bw2, bw3, bw0/bw2, bw3/bw1] (ratios only
    # used on the fast path, which requires |bw1|,|bw2| >> 0)
    fast = []
    wt = np.zeros((R, 6), dtype=np.float64)
    for r in range(R):
        b = bw[:, r].astype(np.float64)
        mx = np.abs(b).max()
        ok = mx > 0 and min(abs(b[1]), abs(b[2])) > 1e-4 * mx
        fast.append(bool(ok))
        wt[r, 0:4] = b
        if ok:
            wt[r, 4] = b[0] / b[2]
            wt[r, 5] = b[3] / b[1]
    wt = np.tile(wt.reshape(1, 6 * R).astype(np.float32), (P, 1))
    return bh, wt, tuple(fast)


_PROGRAM_CACHE = {}


def _get_program(shape, R, fast):
    key = (shape, R, fast)
    if key not in _PROGRAM_CACHE:
        N, C, H, W = shape
        nb = N // N_CORES
        nc = bacc.Bacc(
            "TRN2", target_bir_lowering=False, debug=False, num_devices=N_CORES
        )
        _build_blur_program(nc, nb, C, H, W, R, fast)
        nc.compile()
        _PROGRAM_CACHE[key] = nc
    return _PROGRAM_CACHE[key]


def _run(x, f, trace=False, tmpdir=None):
    x = np.ascontiguousarray(x, dtype=np.float32)
    f = np.ascontiguousarray(f, dtype=np.float32)
    N, C, H, W = x.shape
    OH, OW = H // 2, W // 2
    assert N % N_CORES == 0, f"batch {N} not divisible by {N_CORES} cores"
    nb = N // N_CORES

    R, ah, bw = _factor_filter(f)
    if R == 0:
        return np.zeros((N, C, OH, OW), dtype=np.float32), None
    bh, wt, fast = _build_inputs(ah, bw, H // 2, OH, R)

    nc = _get_program((N, C, H, W), R, fast)
    np_bf16 = mybir.dt.np(BF16)
    # device layout [N, P, C, 2W]: every (partition, channel-group) DMA
    # piece is CG KiB of contiguous DRAM
    xv = np.ascontiguousarray(
        x.reshape(N, C, H // 2, 2 * W).astype(np_bf16).transpose(0, 2, 1, 3)
    )
    bhv = bh.astype(np_bf16)
    in_maps = [
        {"x": xv[k * nb : (k + 1) * nb], "bh": bhv, "wt": wt}
        for k in range(N_CORES)
    ]
    res = run_bass_kernel_spmd(
        nc, in_maps, core_ids=list(range(N_CORES)), trace=trace, tmpdir=tmpdir
    )
    out = np.concatenate(
        [res.results[k]["out"] for k in range(N_CORES)], axis=0
    )
    return out, res


def kernel(x, f):
    out, _ = _run(x, f)
    return out


# revision 30
# speedup vs baseline: 1.2944x; 1.0207x over previous
"""BlurDownsample (depthwise 4x4 FIR + 2x downsample) on 8 TRN2 NeuronCores.

Contract: kernel(x, f) takes the FULL inputs
    x: [16, 128, 256, 256] float32,  f: [4, 4] float32
and returns the FULL output [16, 128, 128, 128] float32, matching
    upfirdn2d(x, f, down=2, padding=(1, 1), flip_filter=False):
    out[n,c,oy,ox] = sum_{dy,dx in 0..3} f[3-dy, 3-dx] * xpad[2oy+dy, 2ox+dx]
with xpad zero-padded by 1 on every spatial edge.

Sharding: pure data-parallel over the batch — core k processes
x[2k:2k+2]; filter-derived constants are replicated.

Per-core strategy (v3 — SDMA-descriptor + HBM-traffic optimized):
  * Host-side, the flipped filter g = flip(f) is factored by SVD into
    R separable terms g = sum_r ah_r (x) bw_r  (R=1 for the
    outer-product filter the model uses).  Only the H-direction runs
    on the Tensor engine; the W-direction is a 4-tap combine on the
    Scalar + Vector engines.  This cuts Tensor-engine streaming 4x
    vs. the banded-matmul-per-filter-column approach.
  * x is converted to bf16 on the host and uploaded as
    [N, C, 128, 512]: HBM read traffic halves (the 2e-2 rel-err gate
    dwarfs bf16's ~2^-9 rounding), every DMA piece is a contiguous
    row pair, and no in-flight cast is needed so loads ride the
    fast HWDGE (sync-engine) path.  SDMA descriptor handling — not
    HBM bandwidth — capped the previous version.
  * The H-FIR+downsample is polyphase banded matmuls in bf16 over
    row-pair partitions: for row parity e, band B_e[p, oh] =
    ah[2p+e-2oh+1] contracts row pairs p, accumulating
    mid[oh, c2, w] in PSUM (2 channels per matmul, rhs free = 512).
    Zero padding in H is implicit in the bands (built host-side).
  * W-combine per channel quad: out[ox] = sum_dx bw[dx]*mid[2ox-1+dx]
    = one Scalar-engine scaled copy (dx=1, full range) plus three
    Vector scalar_tensor_tensor fused multiply-adds (dx=2 full range,
    dx=0/dx=3 edge-clipped), taps as fp32 per-partition SBUF scalars.
  * Stores (fp32) use the scalar-engine HWDGE ring, separate from
    the load ring.
"""

from contextlib import ExitStack

import numpy as np

import concourse.tile as tile
from concourse import bacc, mybir
from concourse.bass_utils import run_bass_kernel_spmd

F32 = mybir.dt.float32
BF16 = mybir.dt.bfloat16

N_CORES = 8
FW = 4  # filter size


def _build_blur_program(nc, N, C, H, W, R, fast):
    OH, OW = H // 2, W // 2
    P = H // 2              # row pairs = SBUF partitions for the contraction
    W2 = 2 * W              # elements per partition row-pair
    CG = min(C, 32)         # channels per load/store group
    QC = 2                  # channels per matmul (PSUM bank: N*4B <= 2KB)
    JJ = min(CG // QC, max(1, 8 // (2 * R)))  # matmul quads per PSUM tile
    assert C % CG == 0 and CG % QC == 0 and P == 128 and W == 256

    x_ap = nc.dram_tensor("x", [N, P, C, W2], BF16, kind="ExternalInput").ap()
    bh_ap = nc.dram_tensor("bh", [R, 2, P, OH], BF16, kind="ExternalInput").ap()
    wt_ap = nc.dram_tensor("wt", [P, 6 * R], F32, kind="ExternalInput").ap()
    out_ap = nc.dram_tensor("out", [N, C, OH, OW], F32, kind="ExternalOutput").ap()

    with tile.TileContext(nc) as tc, ExitStack() as ctx:
        const_pool = ctx.enter_context(tc.tile_pool(name="const", bufs=1))
        x_pool = ctx.enter_context(tc.tile_pool(name="xt", bufs=3))
        acc_pool = ctx.enter_context(tc.tile_pool(name="acc", bufs=4))
        eo_pool = ctx.enter_context(tc.tile_pool(name="eo", bufs=4))
        psum_pool = ctx.enter_context(tc.tile_pool(name="mid", bufs=2, space="PSUM"))

        # ---- one-time setup: load bands + taps ----
        bh_sb = const_pool.tile([P, R, 2, OH], BF16, tag="bh")
        for r in range(R):
            for e in range(2):
                nc.sync.dma_start(out=bh_sb[:, r, e, :], in_=bh_ap[r, e])
        wt_sb = const_pool.tile([P, 6 * R], F32, tag="wt")
        nc.sync.dma_start(out=wt_sb[:, :], in_=wt_ap)

        def wtc(i):
            return wt_sb[:, i : i + 1]

        # ---- main loop: groups of CG channels ----
        for n in range(N):
            for c0 in range(0, C, CG):
                xt = x_pool.tile([P, CG, W2], BF16, tag="xt")
                nc.sync.dma_start(  # host-transposed layout: CG KiB pieces
                    out=xt[:, :, :],
                    in_=x_ap[n, :, c0 : c0 + CG, :],
                )
                for jq in range(CG // (QC * JJ)):
                    acc = acc_pool.tile([OH, JJ, QC, OW], F32, tag="acc")
                    a_full = acc[:, :, :, :]
                    a0 = acc[:, :, :, 1:OW]
                    a3 = acc[:, :, :, 0 : OW - 1]
                    for r in range(R):
                        mid = psum_pool.tile([OH, JJ, QC, W], F32, tag=f"mid{r}")
                        for e in range(2):
                            for jj in range(JJ):
                                c1 = QC * (JJ * jq + jj)
                                nc.tensor.matmul(
                                    mid[:, jj, :, :],
                                    lhsT=bh_sb[:, r, e, :],
                                    rhs=xt[:, c1 : c1 + QC, e * W : (e + 1) * W],
                                    start=(e == 0),
                                    stop=(e == 1),
                                )
                        # W-combine: out[ox] += sum_dx bw[dx]*mid[2ox-1+dx]
                        # with E[k]=mid[2k] (taps 1,3), O[k]=mid[2k+1]
                        # (taps 2,0).
                        if fast[r]:
                            # Scalar engine drains PSUM fast (frees it for
                            # the tensor engine) into scaled polyphases:
                            #   esb = bw1*E, osb = bw2*O.
                            # Vector then combines ALL-SBUF, which enables
                            # the DVE 2x_2p perf mode (fp32 at 2 elem/cyc):
                            #   acc  = esb + osb        (taps 1 and 2)
                            #   acc[1:]   += (bw0/bw2)*osb[:-1]   (tap 0)
                            #   acc[:-1]  += (bw3/bw1)*esb[1:]    (tap 3)
                            esb = eo_pool.tile([OH, JJ, QC, OW], F32, tag="esb")
                            osb = eo_pool.tile([OH, JJ, QC, OW], F32, tag="osb")
                            nc.scalar.mul(
                                esb[:, :, :, :], mid[:, :, :, 0:W:2], wtc(6 * r + 1)
                            )
                            nc.scalar.mul(
                                osb[:, :, :, :], mid[:, :, :, 1:W:2], wtc(6 * r + 2)
                            )
                            if r == 0:
                                nc.vector.scalar_tensor_tensor(
                                    a_full,
                                    esb[:, :, :, :],
                                    1.0,
                                    osb[:, :, :, :],
                                    op0=mybir.AluOpType.mult,
                                    op1=mybir.AluOpType.add,
                                )
                            else:
                                nc.vector.scalar_tensor_tensor(
                                    a_full,
                                    esb[:, :, :, :],
                                    1.0,
                                    a_full,
                                    op0=mybir.AluOpType.mult,
                                    op1=mybir.AluOpType.add,
                                )
                                nc.vector.scalar_tensor_tensor(
                                    a_full,
                                    osb[:, :, :, :],
                                    1.0,
                                    a_full,
                                    op0=mybir.AluOpType.mult,
                                    op1=mybir.AluOpType.add,
                                )
                            nc.vector.scalar_tensor_tensor(
                                a3,
                                esb[:, :, :, 1:OW],
                                wtc(6 * r + 5),
                                a3,
                                op0=mybir.AluOpType.mult,
                                op1=mybir.AluOpType.add,
                            )
                            nc.vector.scalar_tensor_tensor(
                                a0,
                                osb[:, :, :, 0 : OW - 1],
                                wtc(6 * r + 4),
                                a0,
                                op0=mybir.AluOpType.mult,
                                op1=mybir.AluOpType.add,
                            )
                        else:
                            # Degenerate inner taps: combine straight from
                            # PSUM (no ratio trick available).
                            if r == 0:
                                nc.scalar.mul(
                                    a_full, mid[:, :, :, 0:W:2], wtc(6 * r + 1)
                                )
                            else:
                                nc.vector.scalar_tensor_tensor(
                                    a_full,
                                    mid[:, :, :, 0:W:2],
                                    wtc(6 * r + 1),
                                    a_full,
                                    op0=mybir.AluOpType.mult,
                                    op1=mybir.AluOpType.add,
                                )
                            for tap, lo, hi, at in (
                                (2, 1, W, a_full),
                                (0, 1, W - 2, a0),
                                (3, 2, W - 1, a3),
                            ):
                                nc.vector.scalar_tensor_tensor(
                                    at,
                                    mid[:, :, :, lo:hi:2],
                                    wtc(6 * r + tap),
                                    at,
                                    op0=mybir.AluOpType.mult,
                                    op1=mybir.AluOpType.add,
                                )
                    # per-quad store: issued right after this quad's
                    # combine so the scalar queue never blocks more than
                    # one quad deep, and the store stream stays smooth
                    cq = c0 + QC * JJ * jq
                    nc.scalar.dma_start(
                        out=out_ap[n, cq : cq + QC * JJ].rearrange(
                            "c oh ow -> oh c ow"
                        ),
                        in_=acc[:, :, :, :].rearrange("p a b w -> p (a b) w"),
                    )
    return nc


def _factor_filter(f):
    """Factor the flipped filter into R separable (ah, bw) term pairs."""
    g = np.flip(np.asarray(f, dtype=np.float64))
    U, s, Vt = np.linalg.svd(g)
    if s[0] <= 0.0:
        return 0, None, None
    R = int(np.sum(s > s[0] * 1e-4))
    ah = (U[:, :R] * np.sqrt(s[:R])).astype(np.float32)        # [4, R]
    bw = (Vt[:R, :].T * np.sqrt(s[:R])).astype(np.float32)     # [4, R]
    return R, ah, bw


def _build_inputs(ah, bw, P, OH, R):
    bh = np.zeros((R, 2, P, OH), dtype=np.float32)
    for r in range(R):
        for e in range(2):
            for d in range(-2, 3):  # oh = p - d; band is narrow
                dy = 2 * d + e + 1
                if 0 <= dy < FW:
                    idx = np.arange(max(0, d), min(P, OH + d))
                    bh[r, e, idx, idx - d] = ah[dy, r]
    # wt row per r: [bw0, bw1, bw2, bw3, bw0/bw2, bw3/bw1] (ratios only
    # used on the fast path, which requires |bw1|,|bw2| >> 0)
    fast = []
    wt = np.zeros((R, 6), dtype=np.float64)
    for r in range(R):
        b = bw[:, r].astype(np.float64)
        mx = np.abs(b).max()
        ok = mx > 0 and min(abs(b[1]), abs(b[2])) > 1e-4 * mx
        fast.append(bool(ok))
        wt[r, 0:4] = b
        if ok:
            wt[r, 4] = b[0] / b[2]
            wt[r, 5] = b[3] / b[1]
    wt = np.tile(wt.reshape(1, 6 * R).astype(np.float32), (P, 1))
    return bh, wt, tuple(fast)


_PROGRAM_CACHE = {}


def _get_program(shape, R, fast):
    key = (shape, R, fast)
    if key not in _PROGRAM_CACHE:
        N, C, H, W = shape
        nb = N // N_CORES
        nc = bacc.Bacc(
            "TRN2", target_bir_lowering=False, debug=False, num_devices=N_CORES
        )
        _build_blur_program(nc, nb, C, H, W, R, fast)
        nc.compile()
        _PROGRAM_CACHE[key] = nc
    return _PROGRAM_CACHE[key]


def _run(x, f, trace=False, tmpdir=None):
    x = np.ascontiguousarray(x, dtype=np.float32)
    f = np.ascontiguousarray(f, dtype=np.float32)
    N, C, H, W = x.shape
    OH, OW = H // 2, W // 2
    assert N % N_CORES == 0, f"batch {N} not divisible by {N_CORES} cores"
    nb = N // N_CORES

    R, ah, bw = _factor_filter(f)
    if R == 0:
        return np.zeros((N, C, OH, OW), dtype=np.float32), None
    bh, wt, fast = _build_inputs(ah, bw, H // 2, OH, R)

    nc = _get_program((N, C, H, W), R, fast)
    np_bf16 = mybir.dt.np(BF16)
    # device layout [N, P, C, 2W]: every (partition, channel-group) DMA
    # piece is CG KiB of contiguous DRAM
    xv = np.ascontiguousarray(
        x.reshape(N, C, H // 2, 2 * W).astype(np_bf16).transpose(0, 2, 1, 3)
    )
    bhv = bh.astype(np_bf16)
    in_maps = [
        {"x": xv[k * nb : (k + 1) * nb], "bh": bhv, "wt": wt}
        for k in range(N_CORES)
    ]
    res = run_bass_kernel_spmd(
        nc, in_maps, core_ids=list(range(N_CORES)), trace=trace, tmpdir=tmpdir
    )
    out = np.concatenate(
        [res.results[k]["out"] for k in range(N_CORES)], axis=0
    )
    return out, res


def kernel(x, f):
    out, _ = _run(x, f)
    return out


# revision 34
# speedup vs baseline: 1.3794x; 1.0656x over previous
"""BlurDownsample (depthwise 4x4 FIR + 2x downsample) on 8 TRN2 NeuronCores.

Contract: kernel(x, f) takes the FULL inputs
    x: [16, 128, 256, 256] float32,  f: [4, 4] float32
and returns the FULL output [16, 128, 128, 128] float32, matching
    upfirdn2d(x, f, down=2, padding=(1, 1), flip_filter=False):
    out[n,c,oy,ox] = sum_{dy,dx in 0..3} f[3-dy, 3-dx] * xpad[2oy+dy, 2ox+dx]
with xpad zero-padded by 1 on every spatial edge.

Sharding: pure data-parallel over the batch — core k processes
x[2k:2k+2]; filter-derived constants are replicated.

Per-core strategy (v3 — SDMA-descriptor + HBM-traffic optimized):
  * Host-side, the flipped filter g = flip(f) is factored by SVD into
    R separable terms g = sum_r ah_r (x) bw_r  (R=1 for the
    outer-product filter the model uses).  Only the H-direction runs
    on the Tensor engine; the W-direction is a 4-tap combine on the
    Scalar + Vector engines.  This cuts Tensor-engine streaming 4x
    vs. the banded-matmul-per-filter-column approach.
  * x is converted to bf16 on the host and uploaded as
    [N, C, 128, 512]: HBM read traffic halves (the 2e-2 rel-err gate
    dwarfs bf16's ~2^-9 rounding), every DMA piece is a contiguous
    row pair, and no in-flight cast is needed so loads ride the
    fast HWDGE (sync-engine) path.  SDMA descriptor handling — not
    HBM bandwidth — capped the previous version.
  * The H-FIR+downsample is polyphase banded matmuls in bf16 over
    row-pair partitions: for row parity e, band B_e[p, oh] =
    ah[2p+e-2oh+1] contracts row pairs p, accumulating
    mid[oh, c2, w] in PSUM (2 channels per matmul, rhs free = 512).
    Zero padding in H is implicit in the bands (built host-side).
  * W-combine per channel quad: out[ox] = sum_dx bw[dx]*mid[2ox-1+dx]
    = one Scalar-engine scaled copy (dx=1, full range) plus three
    Vector scalar_tensor_tensor fused multiply-adds (dx=2 full range,
    dx=0/dx=3 edge-clipped), taps as fp32 per-partition SBUF scalars.
  * Stores (fp32) use the scalar-engine HWDGE ring, separate from
    the load ring.
"""

from contextlib import ExitStack

import numpy as np

import concourse.tile as tile
from concourse import bacc, mybir
from concourse.bass_utils import run_bass_kernel_spmd

F32 = mybir.dt.float32
BF16 = mybir.dt.bfloat16

N_CORES = 8
FW = 4  # filter size


def _build_blur_program(nc, N, C, H, W, R, fast):
    OH, OW = H // 2, W // 2
    P = H // 2              # row pairs = SBUF partitions for the contraction
    W2 = 2 * W              # elements per partition row-pair
    QC = 2                  # channels per matmul (PSUM bank: N*4B <= 2KB)
    JJ = max(1, 8 // (2 * R))   # matmuls per PSUM tile (PSUM = 2*R tiles)
    CW = QC * JJ            # channels per unit of work (load/combine/store)
    assert C % CW == 0 and P == 128 and W == 256

    x_ap = nc.dram_tensor("x", [N, P, C, W2], BF16, kind="ExternalInput").ap()
    bh_ap = nc.dram_tensor("bh", [R, 2, P, OH], BF16, kind="ExternalInput").ap()
    wt_ap = nc.dram_tensor("wt", [P, 6 * R], F32, kind="ExternalInput").ap()
    out_ap = nc.dram_tensor("out", [N, C, OH, OW], F32, kind="ExternalOutput").ap()

    with tile.TileContext(nc) as tc, ExitStack() as ctx:
        const_pool = ctx.enter_context(tc.tile_pool(name="const", bufs=1))
        x_pool = ctx.enter_context(tc.tile_pool(name="xt", bufs=8))
        acc_pool = ctx.enter_context(tc.tile_pool(name="acc", bufs=4))
        eo_pool = ctx.enter_context(tc.tile_pool(name="eo", bufs=4))
        psum_pool = ctx.enter_context(tc.tile_pool(name="mid", bufs=2, space="PSUM"))

        # ---- one-time setup: load bands + taps ----
        bh_sb = const_pool.tile([P, R, 2, OH], BF16, tag="bh")
        for r in range(R):
            for e in range(2):
                nc.sync.dma_start(out=bh_sb[:, r, e, :], in_=bh_ap[r, e])
        wt_sb = const_pool.tile([P, 6 * R], F32, tag="wt")
        nc.sync.dma_start(out=wt_sb[:, :], in_=wt_ap)

        def wtc(i):
            return wt_sb[:, i : i + 1]

        # ---- main loop: uniform per-quad work units of CW channels ----
        for n in range(N):
            for c0 in range(0, C, CW):
                xt = x_pool.tile([P, CW, W2], BF16, tag="xt")
                nc.sync.dma_start(  # host-transposed layout: CW KiB pieces
                    out=xt[:, :, :],
                    in_=x_ap[n, :, c0 : c0 + CW, :],
                )
                if True:
                    acc = acc_pool.tile([OH, JJ, QC, OW], F32, tag="acc")
                    a_full = acc[:, :, :, :]
                    a0 = acc[:, :, :, 1:OW]
                    a3 = acc[:, :, :, 0 : OW - 1]
                    for r in range(R):
                        mid = psum_pool.tile([OH, JJ, QC, W], F32, tag=f"mid{r}")
                        for e in range(2):
                            for jj in range(JJ):
                                nc.tensor.matmul(
                                    mid[:, jj, :, :],
                                    lhsT=bh_sb[:, r, e, :],
                                    rhs=xt[
                                        :,
                                        QC * jj : QC * (jj + 1),
                                        e * W : (e + 1) * W,
                                    ],
                                    start=(e == 0),
                                    stop=(e == 1),
                                )
                        # W-combine: out[ox] += sum_dx bw[dx]*mid[2ox-1+dx]
                        # with E[k]=mid[2k] (taps 1,3), O[k]=mid[2k+1]
                        # (taps 2,0).
                        if fast[r]:
                            # Scalar engine drains PSUM fast (frees it for
                            # the tensor engine) into scaled polyphases:
                            #   esb = bw1*E, osb = bw2*O.
                            # Vector then combines ALL-SBUF, which enables
                            # the DVE 2x_2p perf mode (fp32 at 2 elem/cyc):
                            #   acc  = esb + osb        (taps 1 and 2)
                            #   acc[1:]   += (bw0/bw2)*osb[:-1]   (tap 0)
                            #   acc[:-1]  += (bw3/bw1)*esb[1:]    (tap 3)
                            esb = eo_pool.tile([OH, JJ, QC, OW], F32, tag="esb")
                            osb = eo_pool.tile([OH, JJ, QC, OW], F32, tag="osb")
                            nc.scalar.mul(
                                esb[:, :, :, :], mid[:, :, :, 0:W:2], wtc(6 * r + 1)
                            )
                            nc.scalar.mul(
                                osb[:, :, :, :], mid[:, :, :, 1:W:2], wtc(6 * r + 2)
                            )
                            if r == 0:
                                nc.vector.scalar_tensor_tensor(
                                    a_full,
                                    esb[:, :, :, :],
                                    1.0,
                                    osb[:, :, :, :],
                                    op0=mybir.AluOpType.mult,
                                    op1=mybir.AluOpType.add,
                                )
                            else:
                                nc.vector.scalar_tensor_tensor(
                                    a_full,
                                    esb[:, :, :, :],
                                    1.0,
                                    a_full,
                                    op0=mybir.AluOpType.mult,
                                    op1=mybir.AluOpType.add,
                                )
                                nc.vector.scalar_tensor_tensor(
                                    a_full,
                                    osb[:, :, :, :],
                                    1.0,
                                    a_full,
                                    op0=mybir.AluOpType.mult,
                                    op1=mybir.AluOpType.add,
                                )
                            nc.vector.scalar_tensor_tensor(
                                a3,
                                esb[:, :, :, 1:OW],
                                wtc(6 * r + 5),
                                a3,
                                op0=mybir.AluOpType.mult,
                                op1=mybir.AluOpType.add,
                            )
                            nc.vector.scalar_tensor_tensor(
                                a0,
                                osb[:, :, :, 0 : OW - 1],
                                wtc(6 * r + 4),
                                a0,
                                op0=mybir.AluOpType.mult,
                                op1=mybir.AluOpType.add,
                            )
                        else:
                            # Degenerate inner taps: combine straight from
                            # PSUM (no ratio trick available).
                            if r == 0:
                                nc.scalar.mul(
                                    a_full, mid[:, :, :, 0:W:2], wtc(6 * r + 1)
                                )
                            else:
                                nc.vector.scalar_tensor_tensor(
                                    a_full,
                                    mid[:, :, :, 0:W:2],
                                    wtc(6 * r + 1),
                                    a_full,
                                    op0=mybir.AluOpType.mult,
                                    op1=mybir.AluOpType.add,
                                )
                            for tap, lo, hi, at in (
                                (2, 1, W, a_full),
                                (0, 1, W - 2, a0),
                                (3, 2, W - 1, a3),
                            ):
                                nc.vector.scalar_tensor_tensor(
                                    at,
                                    mid[:, :, :, lo:hi:2],
                                    wtc(6 * r + tap),
                                    at,
                                    op0=mybir.AluOpType.mult,
                                    op1=mybir.AluOpType.add,
                                )
                    # per-quad store: issued right after this quad's
                    # combine so the scalar queue never blocks more than
                    # one quad deep, and the store stream stays smooth
                    nc.scalar.dma_start(
                        out=out_ap[n, c0 : c0 + CW].rearrange(
                            "c oh ow -> oh c ow"
                        ),
                        in_=acc[:, :, :, :].rearrange("p a b w -> p (a b) w"),
                    )
    return nc


def _factor_filter(f):
    """Factor the flipped filter into R separable (ah, bw) term pairs."""
    g = np.flip(np.asarray(f, dtype=np.float64))
    U, s, Vt = np.linalg.svd(g)
    if s[0] <= 0.0:
        return 0, None, None
    R = int(np.sum(s > s[0] * 1e-4))
    ah = (U[:, :R] * np.sqrt(s[:R])).astype(np.float32)        # [4, R]
    bw = (Vt[:R, :].T * np.sqrt(s[:R])).astype(np.float32)     # [4, R]
    return R, ah, bw


def _build_inputs(ah, bw, P, OH, R):
    bh = np.zeros((R, 2, P, OH), dtype=np.float32)
    for r in range(R):
        for e in range(2):
            for d in range(-2, 3):  # oh = p - d; band is narrow
                dy = 2 * d + e + 1
                if 0 <= dy < FW:
                    idx = np.arange(max(0, d), min(P, OH + d))
                    bh[r, e, idx, idx - d] = ah[dy, r]
    # wt row per r: [bw0, bw1, bw2, bw3, bw0/bw2, bw3/bw1] (ratios only
    # used on the fast path, which requires |bw1|,|bw2| >> 0)
    fast = []
    wt = np.zeros((R, 6), dtype=np.float64)
    for r in range(R):
        b = bw[:, r].astype(np.float64)
        mx = np.abs(b).max()
        ok = mx > 0 and min(abs(b[1]), abs(b[2])) > 1e-4 * mx
        fast.append(bool(ok))
        wt[r, 0:4] = b
        if ok:
            wt[r, 4] = b[0] / b[2]
            wt[r, 5] = b[3] / b[1]
    wt = np.tile(wt.reshape(1, 6 * R).astype(np.float32), (P, 1))
    return bh, wt, tuple(fast)


_PROGRAM_CACHE = {}


def _get_program(shape, R, fast):
    key = (shape, R, fast)
    if key not in _PROGRAM_CACHE:
        N, C, H, W = shape
        nb = N // N_CORES
        nc = bacc.Bacc(
            "TRN2", target_bir_lowering=False, debug=False, num_devices=N_CORES
        )
        _build_blur_program(nc, nb, C, H, W, R, fast)
        nc.compile()
        _PROGRAM_CACHE[key] = nc
    return _PROGRAM_CACHE[key]


def _run(x, f, trace=False, tmpdir=None):
    x = np.ascontiguousarray(x, dtype=np.float32)
    f = np.ascontiguousarray(f, dtype=np.float32)
    N, C, H, W = x.shape
    OH, OW = H // 2, W // 2
    assert N % N_CORES == 0, f"batch {N} not divisible by {N_CORES} cores"
    nb = N // N_CORES

    R, ah, bw = _factor_filter(f)
    if R == 0:
        return np.zeros((N, C, OH, OW), dtype=np.float32), None
    bh, wt, fast = _build_inputs(ah, bw, H // 2, OH, R)

    nc = _get_program((N, C, H, W), R, fast)
    np_bf16 = mybir.dt.np(BF16)
    # device layout [N, P, C, 2W]: every (partition, channel-group) DMA
    # piece is CG KiB of contiguous DRAM
    xv = np.ascontiguousarray(
        x.reshape(N, C, H // 2, 2 * W).astype(np_bf16).transpose(0, 2, 1, 3)
    )
    bhv = bh.astype(np_bf16)
    in_maps = [
        {"x": xv[k * nb : (k + 1) * nb], "bh": bhv, "wt": wt}
        for k in range(N_CORES)
    ]
    res = run_bass_kernel_spmd(
        nc, in_maps, core_ids=list(range(N_CORES)), trace=trace, tmpdir=tmpdir
    )
    out = np.concatenate(
        [res.results[k]["out"] for k in range(N_CORES)], axis=0
    )
    return out, res


def kernel(x, f):
    out, _ = _run(x, f)
    return out
